# revision 1
# baseline (speedup 1.0000x reference)
"""BailingMoe (T=1024, H=1024, E=16, K=4, I=1408, IS=2816) on 8 TRN2 cores.

Strategy: expert-parallel, 2 experts per core, balanced pairing (largest
expert with smallest). The router (x @ w_gate, softmax, top-4, renorm -
0.02% of FLOPs) runs on host as part of input sharding: tokens are
gathered per expert into two capacity slots sized EXACTLY to the max
big / max small expert load (no rounding - matmul free dims are
arbitrary). Each core computes its two experts' MLPs on their gathered
tokens (bf16 matmuls, f32 PSUM accumulation) and a tensor-parallel
slice (IS/8 = 352, zero-padded to 384) of the shared expert over all
tokens. Host unshards: scatter-add the weighted expert outputs and sum
the 8 shared-expert partials.

v2 (after trace analysis of the 152us baseline):
- Routed DOWN is flipped: output tiles are [128 h x tokens] (tokens on
  the matmul free dim), so PE time scales with the real token count
  instead of paying full 512-row instructions for ragged token tiles.
  Combine weights are applied as a host-replicated [128, CS] broadcast
  tile (f32) with one tensor_mul per output tile; output DRAM layout is
  [H, CS+T] (host transposes on unshard).
- gate+up weight tiles ship as one DMA per col-tile ([128,2,8,128]).
- Engine queues balanced: sync=weights, gpsimd=down-weights, scalar=
  activations+inputs, vector=silu-mul + output DMAs (same-queue
  producer->DMA pairs avoid cross-engine semaphores).
- Final phase (slot1 down, second H-half) runs bank-major with weights
  preloaded during the shared-down phase, so banks drain staggered and
  only ~1us of epilogue trails the last matmul.
"""

import functools

import numpy as np
import ml_dtypes

T = 1024
H = 1024
E = 16
K = 4
I = 1408
IS = 2816
ISP = 384          # padded per-core shared-expert slice (2816/8 = 352 -> 384)
TI = I // 128      # 11 intermediate col/row tiles per routed expert
N_CORES = 8

BF16 = ml_dtypes.bfloat16


def _build_nc(C0: int, C1: int):
    import concourse.bass as bass  # noqa: F401  (bacc needs bass loaded)
    import concourse.mybir as mybir
    import concourse.tile as tile
    from concourse import bacc

    BF = mybir.dt.bfloat16
    F32 = mybir.dt.float32
    CS = C0 + C1

    nc = bacc.Bacc(None, target_bir_lowering=False, debug=False)

    # All bulk inputs are pre-tiled on host so every DMA is contiguous.
    xT_ext = nc.declare_dram_parameter("xT", [2, 128, 8, 512], BF, isOutput=False)
    xe0_ext = nc.declare_dram_parameter("xe0", [128, 8, C0], BF, isOutput=False)
    xe1_ext = nc.declare_dram_parameter("xe1", [128, 8, C1], BF, isOutput=False)
    wtb_ext = nc.declare_dram_parameter("wtb", [128, CS], F32, isOutput=False)
    wgu_ext = nc.declare_dram_parameter(
        "w_gu", [2, TI, 128, 2, 8, 128], BF, isOutput=False
    )
    wdn_ext = nc.declare_dram_parameter(
        "w_dn", [2, 2, TI, 128, 512], BF, isOutput=False
    )
    wsgu_ext = nc.declare_dram_parameter(
        "w_sgu", [3, 128, 2, 8, 128], BF, isOutput=False
    )
    wsd_ext = nc.declare_dram_parameter("w_sd", [2, 128, 3, 512], BF, isOutput=False)
    out_ext = nc.declare_dram_parameter("out", [H, CS + T], BF, isOutput=True)

    SILU = mybir.ActivationFunctionType.Silu

    with tile.TileContext(nc) as tc:
        with (
            tc.tile_pool(name="xpool", bufs=1) as xpool,
            tc.tile_pool(name="wp", bufs=10) as wp,
            tc.tile_pool(name="wdn_pool", bufs=6) as wdn_pool,
            tc.tile_pool(name="wdnp_pool", bufs=11) as wdnp_pool,
            tc.tile_pool(name="wsd_pool", bufs=2) as wsd_pool,
            tc.tile_pool(name="a0_pool", bufs=1) as a0_pool,
            tc.tile_pool(name="a1_pool", bufs=1) as a1_pool,
            tc.tile_pool(name="spool", bufs=1) as spool,
            tc.tile_pool(name="tmp_pool", bufs=3) as tmp_pool,
            tc.tile_pool(name="y_pool", bufs=6) as y_pool,
            tc.tile_pool(name="wtbp", bufs=1) as wtbp,
            tc.tile_pool(name="pg", bufs=2, space="PSUM") as pg,
            tc.tile_pool(name="pd", bufs=4, space="PSUM") as pd,
        ):
            # ---- urgent inputs only; everything else is paced into the
            # schedule so the DMA engine (which fair-shares all queued
            # transfers) delivers first-needed data first.
            warm = xpool.tile([128, 512], BF, tag="warm")
            nc.vector.memset(warm[:], 0.0)
            wsgu0 = wp.tile([128, 2, 8, 128], BF, tag="wgu", name="wsgu0")
            nc.sync.dma_start(wsgu0[:], wsgu_ext[0])
            xsb = []
            for t in range(2):
                xt = xpool.tile([128, 8, 512], BF, tag=f"xT{t}", name=f"xt{t}")
                xsb.append(xt)
            for t in range(2):
                nc.scalar.dma_start(xsb[t][:], xT_ext[t])
            xesb0 = xpool.tile([128, 8, C0], BF, tag="xe0")
            xesb1 = xpool.tile([128, 8, C1], BF, tag="xe1")
            wtb_sb = wtbp.tile([128, CS], F32, tag="wtb")

            acte0 = a0_pool.tile([128, TI, C0], BF, tag="a0")
            acte1 = a1_pool.tile([128, TI, C1], BF, tag="a1")
            acts = spool.tile([128, 3, 1024], BF, tag="acts")

            # ---- PE pre-warm: matmuls on zero data release the HAM clock
            # gate while the first input DMAs stream.
            pwarm = pd.tile([128, 512], F32, tag="pd", name="pwarm")
            for _ in range(12):
                nc.tensor.matmul(pwarm, warm[:, :128], warm[:], start=True, stop=True)

            # routed gate/up weight tiles, filled by prefetch callbacks
                wgu_tiles = ([], [])

            def prefetch_wgu(s):
                k = len(wgu_tiles[s])
                if k < TI:
                    wt = wp.tile([128, 2, 8, 128], BF, tag="wgu", name="wpre")
                    eng = nc.gpsimd if s == 0 else nc.sync
                    eng.dma_start(wt[:], wgu_ext[s, k])
                    wgu_tiles[s].append(wt)

            def sgu_steps():
                """Shared gate_up (j-outer). Weight tiles for j=1,2 stream
                on sync at the j boundaries; xe0/xe1/wtb trickle out on the
                scalar queue behind the silus. Each yield paces one slot0
                routed weight tile on gpsimd."""
                for j in range(3):
                    if j == 0:
                        wt = wsgu0
                    else:
                        wt = wp.tile([128, 2, 8, 128], BF, tag="wgu")
                        nc.sync.dma_start(wt[:], wsgu_ext[j])
                    if j == 1:
                        nc.scalar.dma_start(xesb1[:], xe1_ext[:])
                    if j == 2:
                        nc.scalar.dma_start(wtb_sb[:], wtb_ext[:])
                    for tch in range(2):
                        psg = pg.tile([128, 512], F32, tag="psg", name="psg")
                        psu = pg.tile([128, 512], F32, tag="psu", name="psu")
                        prefetch_wgu(0)
                        for h in range(8):
                            nc.tensor.matmul(
                                psg, wt[:, 0, h, :], xsb[tch][:, h, :],
                                start=(h == 0), stop=(h == 7),
                            )
                        yield
                        prefetch_wgu(0)
                        if j == 0 and tch == 1:
                            nc.scalar.dma_start(xesb0[:], xe0_ext[:])
                        for h in range(8):
                            nc.tensor.matmul(
                                psu, wt[:, 1, h, :], xsb[tch][:, h, :],
                                start=(h == 0), stop=(h == 7),
                            )
                        tmp = tmp_pool.tile([128, 512], F32, tag="tmp", name="tmp")
                        nc.scalar.activation(tmp, psg, SILU)
                        nc.vector.tensor_mul(
                            acts[:, j, tch * 512 : (tch + 1) * 512], tmp, psu
                        )
                        yield

            wts_sd = []

            def sdn_steps():
                """Shared down (flipped), pd banks only (pg belongs to the
                woven gate_up). 4 io-outer passes; epilogues split across
                vector+sync and scalar queues."""
                for hh in range(2):
                    w = wsd_pool.tile([128, 3, 512], BF, tag="wsd", name="wsd")
                    nc.gpsimd.dma_start(w[:], wsd_ext[hh])
                    wts_sd.append(w)
                for hh, tch in ((0, 0), (0, 1), (1, 0), (1, 1)):
                    banks = [
                        pd.tile([128, 512], F32, tag="pd", name="pd")
                        for _ in range(4)
                    ]
                    for io in range(3):
                        for b in range(4):
                            nc.tensor.matmul(
                                banks[b],
                                wts_sd[hh][:, io, b * 128 : (b + 1) * 128],
                                acts[:, io, tch * 512 : (tch + 1) * 512],
                                start=(io == 0),
                                stop=(io == 2),
                            )
                        yield
                    for b in range(4):
                        y = y_pool.tile([128, 512], BF, tag="y", name="y")
                        if b % 2 == 0:
                            nc.vector.tensor_copy(y, banks[b])
                            deng = nc.sync
                        else:
                            nc.scalar.copy(y, banks[b])
                            deng = nc.scalar
                        deng.dma_start(
                            out_ext[
                                hh * 512 + b * 128 : hh * 512 + (b + 1) * 128,
                                CS + tch * 512 : CS + (tch + 1) * 512,
                            ],
                            y,
                        )

            def gate_up_steps(s, xes, Cc, a):
                """Two yields per col-tile i: psg chain, then psu chain +
                silu(g)*u epilogue. Weights were prefetched into
                wgu_tiles[s]; slot0 paces slot1's prefetch."""
                for i in range(TI):
                    wt = wgu_tiles[s][i]
                    psg = pg.tile([128, 512], F32, tag="psg", name="psg")[:, :Cc]
                    psu = pg.tile([128, 512], F32, tag="psu", name="psu")[:, :Cc]
                    if s == 0:
                        prefetch_wgu(1)
                    for h in range(8):
                        nc.tensor.matmul(
                            psg, wt[:, 0, h, :], xes[:, h, :],
                            start=(h == 0), stop=(h == 7),
                        )
                    yield
                    for h in range(8):
                        nc.tensor.matmul(
                            psu, wt[:, 1, h, :], xes[:, h, :],
                            start=(h == 0), stop=(h == 7),
                        )
                    tmp = tmp_pool.tile([128, 512], F32, tag="tmp", name="tmp")[
                        :, :Cc
                    ]
                    nc.scalar.activation(tmp, psg, SILU)
                    nc.vector.tensor_mul(a[:, i, :], tmp, psu)
                    yield

            def down_steps(s, a, Cc, cb, hhs, preload_out=None):
                """Flipped down: psum [128 h x Cc tokens], 4 h8-banks per
                H-half, accumulated over the TI intermediate tiles. When
                preload_out is given, also streams the slot1/hh1 tiles for
                the final bank-major pass (one per step)."""
                for hh in hhs:
                    banks = [
                        pd.tile([128, 512], F32, tag="pd", name="pd")[:, :Cc]
                        for _ in range(4)
                    ]
                    for io in range(TI):
                        wdn = wdn_pool.tile([128, 512], BF, tag="wdn", name="wdn")
                        nc.gpsimd.dma_start(wdn[:], wdn_ext[s, hh, io])
                        for b in range(4):
                            nc.tensor.matmul(
                                banks[b],
                                wdn[:, b * 128 : (b + 1) * 128],
                                a[:, io, :],
                                start=(io == 0),
                                stop=(io == TI - 1),
                            )
                        if preload_out is not None and len(preload_out) < TI:
                            wpre = wdnp_pool.tile(
                                [128, 512], BF, tag="wpre", name="wpre"
                            )
                            nc.gpsimd.dma_start(
                                wpre[:], wdn_ext[1, 1, len(preload_out)]
                            )
                            preload_out.append(wpre)
                        yield
                    for b in range(4):
                        y = y_pool.tile([128, 512], BF, tag="y", name="y")[:, :Cc]
                        nc.vector.tensor_mul(y, banks[b], wtb_sb[:, cb : cb + Cc])
                        nc.sync.dma_start(
                            out_ext[
                                hh * 512 + b * 128 : hh * 512 + (b + 1) * 128,
                                cb : cb + Cc,
                            ],
                            y,
                        )

            def dn1_final(wdnt, a, Cc, cb):
                """Slot1 down, second H-half, bank-major with preloaded
                weights: banks drain staggered so only one epilogue trails
                the last matmul."""
                for b in range(4):
                    t = ("psg", "psu", "psg", "psu")[b]
                    ps = pg.tile([128, 512], F32, tag=t, name=t)[:, :Cc]
                    for io in range(TI):
                        nc.tensor.matmul(
                            ps,
                            wdnt[io][:, b * 128 : (b + 1) * 128],
                            a[:, io, :],
                            start=(io == 0),
                            stop=(io == TI - 1),
                        )
                    y = y_pool.tile([128, 512], BF, tag="y", name="y")[:, :Cc]
                    nc.vector.tensor_mul(y, ps, wtb_sb[:, cb : cb + Cc])
                    nc.sync.dma_start(
                        out_ext[
                            512 + b * 128 : 512 + (b + 1) * 128, cb : cb + Cc
                        ],
                        y,
                    )
                    yield

            def drain(g):
                for _ in g:
                    pass

            def weave(g1, g2, ratio, g2_delay=0):
                """Interleave: per g1 step, ~ratio g2 steps; g2 joins
                after g2_delay g1 steps."""
                done1 = done2 = False
                acc = -float(g2_delay) * ratio
                while not (done1 and done2):
                    if not done1:
                        try:
                            next(g1)
                        except StopIteration:
                            done1 = True
                    acc += ratio
                    while acc >= 1.0 and not done2:
                        try:
                            next(g2)
                        except StopIteration:
                            done2 = True
                        acc -= 1.0
                    if done1 and not done2:
                        acc = 1.0

            def chain(*gens):
                for g in gens:
                    yield from g

            # ---- schedule ----
            # Phase AB: shared expert (DMA-light) leads while the routed
            # weights prefetch behind it; GU(0) weaves in as its tiles
            # land. Phase C: GU(1) with all of DN(0) and DN(1,hh0).
            # Phase D: DN(1,hh1) bank-major from preloaded weights.
            drain(sgu_steps())
            weave(gate_up_steps(0, xesb0, C0, acte0), sdn_steps(), 0.55)
            wdnt = []
            weave(
                gate_up_steps(1, xesb1, C1, acte1),
                chain(
                    down_steps(0, acte0, C0, 0, (0, 1)),
                    down_steps(1, acte1, C1, C0, (0,), preload_out=wdnt),
                ),
                1.5,
            )
            drain(dn1_final(wdnt, acte1, C1, C0))

    nc.compile()
    return nc


@functools.lru_cache(maxsize=4)
def _compiled(C0: int, C1: int):
    return _build_nc(C0, C1)


def _route(x, w_gate):
    """Mirror the reference router: softmax, top-4 (desc, ties -> lower
    index), renormalize."""
    logits = x @ w_gate  # f32 [T, E]
    m = logits.max(axis=-1, keepdims=True)
    p = np.exp(logits - m)
    p /= p.sum(axis=-1, keepdims=True)
    order = np.argsort(-p, axis=-1, kind="stable")[:, :K]  # [T, K]
    topw = np.take_along_axis(p, order, axis=-1)
    topw = topw / topw.sum(axis=-1, keepdims=True)
    return order, topw


def kernel(hidden_states, w_gate, w_moe_gate_up, w_moe_down,
           w_shared_gate_up, w_shared_down):
    from concourse.bass_utils import run_bass_kernel_spmd

    x = np.asarray(hidden_states, dtype=np.float32)
    w_gate = np.asarray(w_gate, dtype=np.float32)
    w_moe_gate_up = np.asarray(w_moe_gate_up, dtype=np.float32)
    w_moe_down = np.asarray(w_moe_down, dtype=np.float32)
    w_shared_gate_up = np.asarray(w_shared_gate_up, dtype=np.float32)
    w_shared_down = np.asarray(w_shared_down, dtype=np.float32)

    topk_ids, topk_w = _route(x, w_gate)

    # per-expert token lists + combine weights
    rows_e = []
    wts_e = []
    for e in range(E):
        r, k = np.nonzero(topk_ids == e)
        rows_e.append(r)
        wts_e.append(topk_w[r, k].astype(np.float32))
    counts = np.array([len(r) for r in rows_e])

    # balanced pairing: sort desc; core c gets (big[c], small[c])
    order = np.argsort(-counts, kind="stable")
    slot_experts = [
        (int(order[c]), int(order[2 * N_CORES - 1 - c])) for c in range(N_CORES)
    ]
    C0 = max(16, int(max(counts[a] for a, _ in slot_experts)))
    C1 = max(16, int(max(counts[b] for _, b in slot_experts)))
    CS = C0 + C1

    nc = _compiled(C0, C1)

    xT_bf = np.ascontiguousarray(x.T).astype(BF16)  # [H, T]
    # [H, T] -> [2(tch), 128(p), 8(o), 512]
    xT_t = np.ascontiguousarray(
        xT_bf.reshape(8, 128, 2, 512).transpose(2, 1, 0, 3)
    )
    # [E, H, 2I] -> [E, 11(i), 128(p), 2(g/u), 8(o), 128(c)]
    w_gu_t = np.ascontiguousarray(
        w_moe_gate_up.astype(BF16)
        .reshape(E, 8, 128, 2, TI, 128)
        .transpose(0, 4, 2, 3, 1, 5)
    )
    # [E, I, H] -> [E, 2(hh), 11(io), 128(ip), 512]
    w_dn_t = np.ascontiguousarray(
        w_moe_down.astype(BF16).reshape(E, TI, 128, 2, 512).transpose(0, 3, 1, 2, 4)
    )

    S = IS // N_CORES  # 352
    in_maps = []
    for c in range(N_CORES):
        wtb = np.zeros((CS,), dtype=np.float32)
        wgu = np.empty((2,) + w_gu_t.shape[1:], dtype=BF16)
        wdn = np.empty((2,) + w_dn_t.shape[1:], dtype=BF16)
        xes = []
        for s, e in enumerate(slot_experts[c]):
            cnt = counts[e]
            Cc = (C0, C1)[s]
            b = 0 if s == 0 else C0
            xe = np.zeros((H, Cc), dtype=BF16)
            xe[:, :cnt] = xT_bf[:, rows_e[e]]
            xes.append(
                np.ascontiguousarray(
                    xe.reshape(8, 128, Cc).transpose(1, 0, 2)
                )
            )
            wtb[b : b + cnt] = wts_e[e]
            wgu[s] = w_gu_t[e]
            wdn[s] = w_dn_t[e]
        wsgu = np.zeros((H, 2 * ISP), dtype=BF16)
        wsgu[:, :S] = w_shared_gate_up[:, c * S : (c + 1) * S].astype(BF16)
        wsgu[:, ISP : ISP + S] = w_shared_gate_up[
            :, IS + c * S : IS + (c + 1) * S
        ].astype(BF16)
        # [H, 2*ISP] -> [3(j), 128(p), 2(g/u), 8(o), 128(c)]
        wsgu_t = np.ascontiguousarray(
            wsgu.reshape(8, 128, 2, 3, 128).transpose(3, 1, 2, 0, 4)
        )
        wsd = np.zeros((ISP, H), dtype=BF16)
        wsd[:S] = w_shared_down[c * S : (c + 1) * S].astype(BF16)
        # [ISP, H] -> [2(hh), 128(ip), 3(io), 512]
        wsd_t = np.ascontiguousarray(
            wsd.reshape(3, 128, 2, 512).transpose(2, 1, 0, 3)
        )
        in_maps.append(
            {
                "xT": xT_t,
                "xe0": xes[0],
                "xe1": xes[1],
                "wtb": np.ascontiguousarray(
                    np.broadcast_to(wtb[None, :], (128, CS))
                ),
                "w_gu": wgu,
                "w_dn": wdn,
                "w_sgu": wsgu_t,
                "w_sd": wsd_t,
            }
        )

    res = run_bass_kernel_spmd(nc, in_maps, core_ids=list(range(N_CORES)))

    out = np.zeros((T, H), dtype=np.float32)
    acc = np.zeros((H, T), dtype=np.float32)
    for c in range(N_CORES):
        r = np.asarray(res.results[c]["out"], dtype=np.float32)  # [H, CS+T]
        for s, e in enumerate(slot_experts[c]):
            cnt = counts[e]
            b = 0 if s == 0 else C0
            out[rows_e[e]] += r[:, b : b + cnt].T
        acc += r[:, CS:]
    out += acc.T
    return out

